# revision 1
# baseline (speedup 1.0000x reference)
"""Trainium2 Bass kernel for nn_Attention (B=4, S=2048, D=2048, H=16, KV=4, HD=128).

Sharding (8 cores): data-parallel over batch (4) x tensor-parallel over
KV-head-group halves (2). Core c handles batch b=c//2 and q-heads
[8*(c%2), 8*(c%2)+8) == kv groups {2*(c%2), 2*(c%2)+1}. Each core produces a
partial output (its heads' contribution through wo); the host sums the two
partials per batch.

All big matmuls run in float32r (full PE speed, ~1.6e-4 rel err). Attention is
computed transposed (scoresT[k,q]: kT-block stationary, qT moving) so the ACT
exp pass doubles as the PSUM->SBUF move and no probs transposes are needed (no
max subtraction; scores are O(6) here). Softmax denominators come from a
ones-row matmul accumulated in PSUM; normalization multiplies the AV output by
a broadcast reciprocal tile (ones-column x recip-row matmul). AV accumulates in
PSUM (V stationary, probsT moving); the output projection (woT stationary,
attT moving) emits a transposed partial output; host transposes back and sums
core pairs. Copy engines (ACT vs DVE) and PSUM/SBUF pool depths are tuned via
TimelineSim A/B sweeps: ~589us/core, ~1.21x the fp32r PE-work floor.
"""
import numpy as np

B, S, D = 4, 2048, 2048
H, KV, HD = 16, 4, 128
NREP = H // KV
SCALE = float(HD) ** -0.5

SB = S // 128          # 16 s-blocks
KT = D // 128          # 16 contraction tiles for projections
QSB = S // 512         # 4 q-superblocks
HPC = 8                # q heads per core
GPC = 2                # kv groups per core

_compiled = {}


def _build(causal: bool):
    import concourse.bass as bass  # noqa: F401
    import concourse.tile as tile
    from concourse import bacc, mybir
    from concourse.masks import make_identity

    f32 = mybir.dt.float32
    f32r = mybir.dt.float32r
    AF = mybir.ActivationFunctionType
    ALU = mybir.AluOpType

    nc = bacc.Bacc("TRN2")

    xT = nc.dram_tensor("xT", [D, S], f32r, kind="ExternalInput")
    wqT = nc.dram_tensor("wqT", [D, HPC * HD], f32r, kind="ExternalInput")
    wkvT = nc.dram_tensor("wkvT", [D, 2 * GPC * HD], f32r, kind="ExternalInput")
    woT = nc.dram_tensor("woT", [HPC * HD, D], f32r, kind="ExternalInput")
    cosS = nc.dram_tensor("cosS", [128, SB, 64], f32, kind="ExternalInput")
    sinS = nc.dram_tensor("sinS", [128, SB, 64], f32, kind="ExternalInput")
    mtile = nc.dram_tensor("mtile", [128, 128], f32, kind="ExternalInput")
    onest = nc.dram_tensor("onest", [128, 128], f32r, kind="ExternalInput")
    outT = nc.dram_tensor("outT", [D, S], f32, kind="ExternalOutput")

    xT3 = xT.rearrange("(kt p) s -> p kt s", p=128)
    woT3 = woT.rearrange("(h p) d -> p h d", p=128)

    with tile.TileContext(nc) as tc:
        with tc.tile_pool(name="persist", bufs=1) as persist:
            qT = [persist.tile([128, S], f32r, tag=f"qT{h}", name=f"qT{h}") for h in range(HPC)]
            kT = [persist.tile([128, S], f32r, tag=f"kTg{g}", name=f"kTg{g}") for g in range(GPC)]
            vsb = [persist.tile([128, SB, 128], f32r, tag=f"v{g}", name=f"v{g}") for g in range(GPC)]
            msk = persist.tile([128, 128], f32, tag="msk")
            nc.sync.dma_start(out=msk, in_=mtile[:, :])
            ones = persist.tile([128, 128], f32r, tag="ones")
            nc.sync.dma_start(out=ones, in_=onest[:, :])

            # ------------ Stage 1: projections + RoPE + transposes ----------
            s1ctx = tc.tile_pool(name="s1const", bufs=1)
            s1const = s1ctx.__enter__()
            ident_f = s1const.tile([128, 128], f32, tag="identf")
            make_identity(nc, ident_f)
            ident = s1const.tile([128, 128], f32r, tag="ident")
            nc.vector.tensor_copy(out=ident, in_=ident_f)
            cos_t = s1const.tile([128, SB, 64], f32, tag="cos")
            sin_t = s1const.tile([128, SB, 64], f32, tag="sin")
            nc.sync.dma_start(out=cos_t, in_=cosS[:, :, :])
            nc.sync.dma_start(out=sin_t, in_=sinS[:, :, :])

            def proj_pass(wT_ap, e_width, kind, head_base=0):
                nh = e_width // 128
                with tc.tile_pool(name="w1", bufs=1) as wpool, \
                     tc.tile_pool(name="xs1", bufs=2) as xpool, \
                     tc.tile_pool(name="rs1", bufs=2) as rpool, \
                     tc.tile_pool(name="pq1", bufs=3, space="PSUM") as pqp, \
                     tc.tile_pool(name="pt1", bufs=2, space="PSUM") as ptp:
                    wt = wpool.tile([128, KT, e_width], f32r, tag="wt")
                    wT3 = wT_ap.rearrange("(kt p) e -> p kt e", p=128)
                    for kt4 in range(0, KT, 2):
                        nc.sync.dma_start(
                            out=wt[:, kt4:kt4 + 2, :], in_=wT3[:, kt4:kt4 + 2, :])
                    for sb in range(SB):
                        xs = xpool.tile([128, KT, 128], f32r, tag="xs")
                        nc.sync.dma_start(
                            out=xs[:, 0:8, :],
                            in_=xT3[:, 0:8, sb * 128:(sb + 1) * 128])
                        nc.sync.dma_start(
                            out=xs[:, 8:16, :],
                            in_=xT3[:, 8:16, sb * 128:(sb + 1) * 128])
                        ps = pqp.tile([128, e_width], f32, tag="ps")
                        for kt in range(KT):
                            for n0 in range(0, e_width, 512):
                                nw = min(512, e_width - n0)
                                nc.tensor.matmul(
                                    ps[:, n0:n0 + nw], xs[:, kt, :],
                                    wt[:, kt, n0:n0 + nw],
                                    start=(kt == 0), stop=(kt == KT - 1))
                        ps3 = ps.rearrange("p (h d) -> p h d", d=128)
                        nr = GPC if kind == "kv" else nh  # heads that get RoPE
                        if kind == "kv":
                            for g in range(GPC):
                                nc.scalar.copy(
                                    out=vsb[g][:, sb, :], in_=ps3[:, GPC + g, :])
                        rp = rpool.tile([128, HPC, 128], f32r, tag="rope")
                        ev = ps3[:, 0:nr, 0:128:2]
                        od = ps3[:, 0:nr, 1:128:2]
                        cb = cos_t[:, None, sb, :].broadcast_to([128, nr, 64])
                        sn = sin_t[:, None, sb, :].broadcast_to([128, nr, 64])
                        t1 = rpool.tile([128, HPC, 64], f32, tag="t1")
                        t2 = rpool.tile([128, HPC, 64], f32, tag="t2")
                        nc.vector.tensor_tensor(
                            out=t1[:, 0:nr, :], in0=ev, in1=cb, op=ALU.mult)
                        nc.vector.tensor_tensor(
                            out=t2[:, 0:nr, :], in0=od, in1=sn, op=ALU.mult)
                        nc.vector.tensor_tensor(
                            out=rp[:, 0:nr, 0:64], in0=t1[:, 0:nr, :],
                            in1=t2[:, 0:nr, :], op=ALU.subtract)
                        nc.vector.tensor_tensor(
                            out=t1[:, 0:nr, :], in0=ev, in1=sn, op=ALU.mult)
                        nc.vector.tensor_tensor(
                            out=t2[:, 0:nr, :], in0=od, in1=cb, op=ALU.mult)
                        nc.vector.tensor_tensor(
                            out=rp[:, 0:nr, 64:128], in0=t1[:, 0:nr, :],
                            in1=t2[:, 0:nr, :], op=ALU.add)
                        for h in range(nr):
                            pt = ptp.tile([128, 128], f32r, tag="pt")
                            nc.tensor.transpose(pt, rp[:, h, :], ident)
                            dst = (qT[head_base + h] if kind == "q"
                                   else kT[head_base + h])
                            nc.vector.tensor_copy(
                                out=dst[:, sb * 128:(sb + 1) * 128], in_=pt)

            proj_pass(wkvT[:, :], 2 * GPC * HD, "kv")
            proj_pass(wqT[:, :], HPC * HD, "q", head_base=0)
            s1ctx.__exit__(None, None, None)

            # ------------ Stage 2+3: attention (scoresT) + out-projection ---
            with tc.tile_pool(name="wo2", bufs=1) as wopool, \
                 tc.tile_pool(name="wom2", bufs=2) as womp, \
                 tc.tile_pool(name="pr2", bufs=2) as prpool, \
                 tc.tile_pool(name="att2", bufs=1) as attpool, \
                 tc.tile_pool(name="dn2", bufs=1) as dnpool, \
                 tc.tile_pool(name="o2", bufs=2) as opool, \
                 tc.tile_pool(name="psc", bufs=4, space="PSUM") as pscp, \
                 tc.tile_pool(name="pds", bufs=1, space="PSUM") as pdsp, \
                 tc.tile_pool(name="pav", bufs=2, space="PSUM") as pavp, \
                 tc.tile_pool(name="pou", bufs=1, space="PSUM") as poup:
                for qsb in range(QSB):
                    att = attpool.tile([128, HPC, 512], f32r, tag="att")
                    maxkt = (qsb + 1) * 4 if causal else SB
                    q0g = qsb * 512
                    for g in range(GPC):
                        rr = [dnpool.tile([1, 512], f32r, tag=f"rr{r}",
                                          name=f"rr{r}") for r in range(NREP)]
                        for r in range(NREP):
                            h = g * NREP + r
                            probs = prpool.tile([128, SB, 512], f32r, tag="probs")
                            dsum = pdsp.tile([1, 512], f32, tag="dsum")
                            for t in range(maxkt):
                                # local q start within this superblock
                                ql = max(0, t * 128 - q0g) if causal else 0
                                qw = 512 - ql
                                sc = pscp.tile([128, 512], f32, tag="sc")
                                nc.tensor.matmul(
                                    sc[:, ql:512],
                                    kT[g][:, t * 128:(t + 1) * 128],
                                    qT[h][:, q0g + ql:q0g + 512],
                                    start=True, stop=True)
                                is_diag = causal and t * 128 >= q0g
                                if is_diag:
                                    # add mask pre-scale: exp(SCALE*(sc+msk))
                                    # == exp(SCALE*sc + mask) for the 0/-inf
                                    # mask (underflows to 0 identically)
                                    nc.vector.tensor_tensor(
                                        out=sc[:, ql:ql + 128],
                                        in0=sc[:, ql:ql + 128],
                                        in1=msk, op=ALU.add)
                                nc.scalar.activation(
                                    out=probs[:, t, ql:512],
                                    in_=sc[:, ql:512], func=AF.Exp,
                                    scale=SCALE)
                                nc.tensor.matmul(
                                    dsum[:, ql:512], ones[:, 0:1],
                                    probs[:, t, ql:512],
                                    start=(t == 0), stop=(t == maxkt - 1),
                                    skip_group_check=True)
                                if causal and ql > 0:
                                    # q < k region contributes nothing, but the
                                    # dsum psum slice [0:ql] of t==0 already
                                    # covers it (probs[:,0,0:512] full).
                                    pass
                            # reciprocal row -> R tile via ones-matmul
                            with nc.allow_low_precision(reason="softmax recip"):
                                nc.vector.reciprocal(out=rr[r], in_=dsum)
                            # AV accumulate; normalization happens per group
                            av = pavp.tile([128, 512], f32, tag="av")
                            for t in range(maxkt):
                                ql = max(0, t * 128 - q0g) if causal else 0
                                nc.tensor.matmul(
                                    av[:, ql:512], vsb[g][:, t, :],
                                    probs[:, t, ql:512],
                                    start=(t == 0), stop=(t == maxkt - 1),
                                    skip_group_check=True)
                            nc.vector.tensor_copy(out=att[:, h, :], in_=av)
                        rsb = dnpool.tile([128, 4, 512], f32, tag="rsb")
                        for r in range(NREP):
                            rps = pscp.tile([128, 512], f32, tag="sc")
                            nc.tensor.matmul(
                                rps, ones[0:1, :], rr[r],
                                start=True, stop=True)
                            nc.scalar.copy(out=rsb[:, r, :], in_=rps)
                        for r in range(NREP):
                            h = g * NREP + r
                            nc.vector.tensor_tensor(
                                out=att[:, h, :], in0=att[:, h, :],
                                in1=rsb[:, r, :], op=ALU.mult)
                    # out-projection for this q-superblock
                    for m in range(KT):
                        wom = womp.tile([128, HPC, 128], f32r, tag="wom")
                        nc.sync.dma_start(
                            out=wom, in_=woT3[:, :, m * 128:(m + 1) * 128])
                        wsrc = wom
                        po = poup.tile([128, 512], f32, tag="po")
                        for e in range(HPC):
                            nc.tensor.matmul(
                                po, wsrc[:, e, :], att[:, e, :],
                                start=(e == 0), stop=(e == HPC - 1))
                        ot = opool.tile([128, 512], f32, tag="ot")
                        nc.vector.tensor_copy(out=ot, in_=po)
                        nc.sync.dma_start(
                            out=outT[m * 128:(m + 1) * 128,
                                     qsb * 512:(qsb + 1) * 512],
                            in_=ot)

    nc.compile()
    return nc


def _get_nc(causal: bool):
    if causal not in _compiled:
        _compiled[causal] = _build(causal)
    return _compiled[causal]


def kernel(x, freqs_cis, mask, wq, wk, wv, wo):
    from concourse.bass_utils import run_bass_kernel_spmd

    x = np.asarray(x, dtype=np.float32)
    freqs_cis = np.asarray(freqs_cis, dtype=np.float32)
    mask = np.asarray(mask, dtype=np.float32)
    wq = np.asarray(wq, dtype=np.float32)
    wk = np.asarray(wk, dtype=np.float32)
    wv = np.asarray(wv, dtype=np.float32)
    wo = np.asarray(wo, dtype=np.float32)

    tri = np.tril(np.ones((S, S), dtype=bool))
    causal = bool((mask[tri] == 0.0).all() and (mask[~tri] < -1e30).all())
    if not causal and not (mask == 0.0).all():
        return _numpy_ref(x, freqs_cis, mask, wq, wk, wv, wo)

    nc = _get_nc(causal)

    cos = freqs_cis[:, :, 0]
    sin = freqs_cis[:, :, 1]
    cosS = np.ascontiguousarray(cos.reshape(SB, 128, 64).transpose(1, 0, 2))
    sinS = np.ascontiguousarray(sin.reshape(SB, 128, 64).transpose(1, 0, 2))
    mtile = (np.ascontiguousarray(mask[0:128, 0:128].T) if causal
             else np.zeros((128, 128), dtype=np.float32))
    onest = np.ones((128, 128), dtype=np.float32)

    in_maps = []
    for c in range(8):
        b, i = c // 2, c % 2
        in_maps.append({
            "xT": np.ascontiguousarray(x[b].T),
            "wqT": np.ascontiguousarray(wq[1024 * i:1024 * (i + 1), :].T),
            "wkvT": np.ascontiguousarray(np.concatenate(
                [wk[256 * i:256 * (i + 1), :].T,
                 wv[256 * i:256 * (i + 1), :].T], axis=1)),
            "woT": np.ascontiguousarray(wo[:, 1024 * i:1024 * (i + 1)].T),
            "cosS": cosS, "sinS": sinS, "mtile": mtile, "onest": onest,
        })

    res = run_bass_kernel_spmd(nc, in_maps, core_ids=list(range(8)))
    out = np.empty((B, S, D), dtype=np.float32)
    for b in range(B):
        out[b] = res.results[2 * b]["outT"].T + res.results[2 * b + 1]["outT"].T
    return out


def _numpy_ref(x, freqs_cis, mask, wq, wk, wv, wo):
    xq = (x @ wq.T).reshape(B, S, H, HD)
    xk = (x @ wk.T).reshape(B, S, KV, HD)
    xv = (x @ wv.T).reshape(B, S, KV, HD)

    def rope(xh):
        x2 = xh.reshape(*xh.shape[:-1], HD // 2, 2)
        fc = freqs_cis[None, :, None, :, :]
        real = x2[..., 0] * fc[..., 0] - x2[..., 1] * fc[..., 1]
        imag = x2[..., 0] * fc[..., 1] + x2[..., 1] * fc[..., 0]
        return np.concatenate([real, imag], axis=-1)

    xq, xk = rope(xq), rope(xk)
    q = xq.reshape(B, S, KV, NREP, HD)
    sc = np.einsum('bqgrd,bkgd->bgrqk', q, xk) * SCALE + mask[None, None, None]
    sc = sc - sc.max(axis=-1, keepdims=True)
    p = np.exp(sc)
    p /= p.sum(axis=-1, keepdims=True)
    o = np.einsum('bgrqk,bkgd->bqgrd', p, xv).reshape(B, S, H * HD)
    return (o @ wo.T).astype(np.float32)



# revision 10
# speedup vs baseline: 1.2371x; 1.2371x over previous
"""Trainium2 Bass kernel for nn_Attention (B=4, S=2048, D=2048, H=16, KV=4, HD=128).

Sharding (8 cores): data-parallel over batch (4) x tensor-parallel over
KV-head-group halves (2). Core c handles batch b=c//2 and q-heads
[8*(c%2), 8*(c%2)+8) == kv groups {2*(c%2), 2*(c%2)+1}. Each core produces a
partial output (its heads' contribution through wo); the host sums the two
partials per batch.

v2: Q/K/V projections run as fp8e4 DoubleRow matmuls (0.5 cycles/row, two
128-deep contraction subtiles per instruction) with an error-corrected hi/lo
split of both x and the weights (3 of 4 cross terms; ~2^-8 effective
precision). Weights are pre-scaled x32 on the host so the lo residual clears
e4m3's subnormal floor; the descale folds into cos/sin (q,k) and a 32.0
ones-column in the softmax denominator (v). Attention (scores, exp, AV) and
the output projection run in bf16. Softmax denominators come from
probs-stationary x ones-moving matmuls (out [128q,1], ~1 cycle each instead
of 512); the reciprocal row reaches [dmodel, q] broadcast layout via a tiny
transpose + selector matmuls. Normalization is fused into the PSUM->SBUF
copy of AV. Out-projection chunks are interleaved into the next
q-superblock's attention so the single PSUM bank never stalls the PE.
"""
import numpy as np

B, S, D = 4, 2048, 2048
H, KV, HD = 16, 4, 128
NREP = H // KV
SCALE = float(HD) ** -0.5
WS = 32.0                  # host weight pre-scale (power of 2)

SB = S // 128          # 16 s-blocks
KT = D // 128          # 16 contraction tiles for projections
QSB = S // 512         # 4 q-superblocks
HPC = 8                # q heads per core
GPC = 2                # kv groups per core

_compiled = {}


def _build(causal: bool):
    import concourse.bass as bass  # noqa: F401
    import concourse.tile as tile
    from concourse import bacc, mybir
    from concourse.masks import make_identity

    f32 = mybir.dt.float32
    bf16 = mybir.dt.bfloat16
    f8 = mybir.dt.float8e4
    DR = mybir.MatmulPerfMode.DoubleRow
    AF = mybir.ActivationFunctionType
    ALU = mybir.AluOpType

    nc = bacc.Bacc("TRN2")

    # x hi/lo: [0]=hi, [1]=lo.  weights hi/lo: [0]=LO, [1]=HI (two-major layout)
    xT8 = nc.dram_tensor("xT8", [2, D, S], f8, kind="ExternalInput")
    wq8 = nc.dram_tensor("wq8", [2, D, HPC * HD], f8, kind="ExternalInput")
    wkv8 = nc.dram_tensor("wkv8", [2, D, 2 * GPC * HD], f8, kind="ExternalInput")
    wob = nc.dram_tensor("wob", [HPC * HD, D], bf16, kind="ExternalInput")
    cosS = nc.dram_tensor("cosS", [128, SB, 64], f32, kind="ExternalInput")
    sinS = nc.dram_tensor("sinS", [128, SB, 64], f32, kind="ExternalInput")
    mtile = nc.dram_tensor("mtile", [128, 128], f32, kind="ExternalInput")
    ones32 = nc.dram_tensor("ones32", [128, 1], bf16, kind="ExternalInput")
    selS = nc.dram_tensor("selS", [4, 4 * 128], bf16, kind="ExternalInput")
    outT = nc.dram_tensor("outT", [D, S], f32, kind="ExternalOutput")

    xT4 = xT8.rearrange("two (kt p) s -> p two kt s", p=128)
    wq4 = wq8.rearrange("two (kt p) e -> p two kt e", p=128)
    wkv4 = wkv8.rearrange("two (kt p) e -> p two kt e", p=128)
    wo3 = wob.rearrange("(h p) d -> p h d", p=128)

    with tile.TileContext(nc) as tc:
        with tc.tile_pool(name="persist", bufs=1) as persist:
            qT = [persist.tile([128, S], bf16, tag=f"qT{h}", name=f"qT{h}") for h in range(HPC)]
            kT = [persist.tile([128, S], bf16, tag=f"kTg{g}", name=f"kTg{g}") for g in range(GPC)]
            vsb = [persist.tile([128, SB, 128], bf16, tag=f"v{g}", name=f"v{g}") for g in range(GPC)]
            wos = persist.tile([128, HPC, D], bf16, tag="wos")
            msk = persist.tile([128, 128], f32, tag="msk")
            nc.sync.dma_start(out=msk, in_=mtile[:, :])
            onec = persist.tile([128, 1], bf16, tag="onec")
            nc.sync.dma_start(out=onec, in_=ones32[:, :])
            selt = persist.tile([4, 4, 128], bf16, tag="selt")
            sel3 = selS.rearrange("k (m d) -> k m d", d=128)
            nc.sync.dma_start(out=selt, in_=sel3[:, :, :])
            ident = persist.tile([128, 128], bf16, tag="ident")

            # ------------ Stage 1: projections + RoPE + transposes ----------
            s1ctx = tc.tile_pool(name="s1const", bufs=1)
            s1const = s1ctx.__enter__()
            ident_f = s1const.tile([128, 128], f32, tag="identf")
            make_identity(nc, ident_f)
            nc.vector.tensor_copy(out=ident, in_=ident_f)
            cos_t = s1const.tile([128, SB, 64], f32, tag="cos")
            sin_t = s1const.tile([128, SB, 64], f32, tag="sin")
            nc.sync.dma_start(out=cos_t, in_=cosS[:, :, :])
            nc.sync.dma_start(out=sin_t, in_=sinS[:, :, :])

            def proj_pass(w4, e_width, kind, wos_chunks):
                nh = e_width // 128
                with tc.tile_pool(name="w1", bufs=1) as wpool, \
                     tc.tile_pool(name="xs1", bufs=2) as xpool, \
                     tc.tile_pool(name="rs1", bufs=2) as rpool, \
                     tc.tile_pool(name="pq1", bufs=3, space="PSUM") as pqp, \
                     tc.tile_pool(name="pt1", bufs=2, space="PSUM") as ptp:
                    wt = wpool.tile([128, 2, KT, e_width], f8, tag="wt")
                    for k0 in range(0, KT, 2):
                        for v in range(2):
                            nc.sync.dma_start(
                                out=wt[:, v, k0:k0 + 2, :],
                                in_=w4[:, v, k0:k0 + 2, :])
                    for sb in range(SB):
                        if sb in wos_chunks:
                            c = wos_chunks[sb]
                            nc.sync.dma_start(
                                out=wos[:, :, 512 * c:512 * (c + 1)],
                                in_=wo3[:, :, 512 * c:512 * (c + 1)])
                        xs = xpool.tile([128, 2, KT, 128], f8, tag="xs")
                        for v in range(2):
                            nc.sync.dma_start(
                                out=xs[:, v, 0:8, :],
                                in_=xT4[:, v, 0:8, sb * 128:(sb + 1) * 128])
                            nc.sync.dma_start(
                                out=xs[:, v, 8:16, :],
                                in_=xT4[:, v, 8:16, sb * 128:(sb + 1) * 128])
                        ps = pqp.tile([128, e_width], f32, tag="ps")
                        for n0 in range(0, e_width, 256):
                            for j in range(KT // 2):
                                nc.tensor.matmul(
                                    ps[:, n0:n0 + 256],
                                    xs[:, 0, 2 * j:2 * j + 2, :],
                                    wt[:, 1, 2 * j:2 * j + 2, n0:n0 + 256],
                                    start=(j == 0), stop=False, perf_mode=DR)
                            for kt in range(KT):
                                nc.tensor.matmul(
                                    ps[:, n0:n0 + 256],
                                    xs[:, 0:2, kt, :],
                                    wt[:, 0:2, kt, n0:n0 + 256],
                                    start=False, stop=(kt == KT - 1),
                                    perf_mode=DR)
                        ps3 = ps.rearrange("p (h d) -> p h d", d=128)
                        nr = GPC if kind == "kv" else nh  # heads that get RoPE
                        if kind == "kv":
                            for g in range(GPC):
                                nc.scalar.copy(
                                    out=vsb[g][:, sb, :], in_=ps3[:, GPC + g, :])
                        rp = rpool.tile([128, HPC, 128], bf16, tag="rope")
                        ev = ps3[:, 0:nr, 0:128:2]
                        od = ps3[:, 0:nr, 1:128:2]
                        cb = cos_t[:, None, sb, :].broadcast_to([128, nr, 64])
                        sn = sin_t[:, None, sb, :].broadcast_to([128, nr, 64])
                        t1 = rpool.tile([128, HPC, 64], f32, tag="t1")
                        t2 = rpool.tile([128, HPC, 64], f32, tag="t2")
                        nc.vector.tensor_tensor(
                            out=t1[:, 0:nr, :], in0=ev, in1=cb, op=ALU.mult)
                        nc.vector.tensor_tensor(
                            out=t2[:, 0:nr, :], in0=od, in1=sn, op=ALU.mult)
                        nc.vector.tensor_tensor(
                            out=rp[:, 0:nr, 0:64], in0=t1[:, 0:nr, :],
                            in1=t2[:, 0:nr, :], op=ALU.subtract)
                        nc.vector.tensor_tensor(
                            out=t1[:, 0:nr, :], in0=ev, in1=sn, op=ALU.mult)
                        nc.vector.tensor_tensor(
                            out=t2[:, 0:nr, :], in0=od, in1=cb, op=ALU.mult)
                        nc.vector.tensor_tensor(
                            out=rp[:, 0:nr, 64:128], in0=t1[:, 0:nr, :],
                            in1=t2[:, 0:nr, :], op=ALU.add)
                        for h in range(nr):
                            pt = ptp.tile([128, 1024], bf16, tag="pt")
                            nc.tensor.transpose(
                                pt[:, 0:128], rp[:, h, :], ident)
                            dst = qT[h] if kind == "q" else kT[h]
                            nc.vector.tensor_copy(
                                out=dst[:, sb * 128:(sb + 1) * 128],
                                in_=pt[:, 0:128])

            proj_pass(wkv4, 2 * GPC * HD, "kv", {4: 0, 12: 1})
            proj_pass(wq4, HPC * HD, "q", {4: 2, 12: 3})
            s1ctx.__exit__(None, None, None)

            # ------------ Stage 2+3: attention + out-projection -------------
            with tc.tile_pool(name="pr2", bufs=2) as prpool, \
                 tc.tile_pool(name="att2", bufs=2) as attpool, \
                 tc.tile_pool(name="dn2", bufs=2) as dnpool, \
                 tc.tile_pool(name="o2", bufs=2) as opool, \
                 tc.tile_pool(name="psc", bufs=3, space="PSUM") as pscp, \
                 tc.tile_pool(name="pav", bufs=1, space="PSUM") as pavp, \
                 tc.tile_pool(name="pds", bufs=1, space="PSUM") as pdsp, \
                 tc.tile_pool(name="prt", bufs=1, space="PSUM") as prtp, \
                 tc.tile_pool(name="prs", bufs=1, space="PSUM") as prsp, \
                 tc.tile_pool(name="pou", bufs=1, space="PSUM") as poup:

                pending = []  # (qsb, att_tile, m) out-proj chunks not yet run

                def oproj_chunk(use_psc=False):
                    if not pending:
                        return
                    oq, oatt, m = pending.pop(0)
                    if use_psc:
                        po = pscp.tile([128, 512], f32, tag="sc")
                    else:
                        po = poup.tile([128, 512], f32, tag="po")
                    for e in range(HPC):
                        nc.tensor.matmul(
                            po, wos[:, e, m * 128:(m + 1) * 128],
                            oatt[:, e, :], start=(e == 0), stop=(e == HPC - 1),
                            skip_group_check=True)
                    ot = opool.tile([128, 512], f32, tag="ot")
                    nc.vector.tensor_copy(out=ot, in_=po)
                    nc.sync.dma_start(
                        out=outT[m * 128:(m + 1) * 128,
                                 oq * 512:(oq + 1) * 512],
                        in_=ot)

                for qsb in range(QSB):
                    att = attpool.tile([128, HPC, 512], bf16, tag="att")
                    maxkt = (qsb + 1) * 4 if causal else SB
                    q0g = qsb * 512
                    for g in range(GPC):
                        pds = pdsp.tile([128, 512], f32, tag="pds")
                        for r in range(NREP):
                            h = g * NREP + r
                            probs = prpool.tile([128, SB, 512], bf16, tag="probs")
                            for t in range(maxkt):
                                ql = max(0, t * 128 - q0g) if causal else 0
                                sc = pscp.tile([128, 512], f32, tag="sc")
                                nc.tensor.matmul(
                                    sc[:, ql:512],
                                    kT[g][:, t * 128:(t + 1) * 128],
                                    qT[h][:, q0g + ql:q0g + 512],
                                    start=True, stop=True)
                                if causal and t * 128 >= q0g:
                                    nc.vector.tensor_tensor(
                                        out=sc[:, ql:ql + 128],
                                        in0=sc[:, ql:ql + 128],
                                        in1=msk, op=ALU.add)
                                nc.scalar.activation(
                                    out=probs[:, t, ql:512],
                                    in_=sc[:, ql:512], func=AF.Exp,
                                    scale=SCALE)
                            oproj_chunk()
                            # AV accumulation (x WS via v scaling)
                            av = pavp.tile([128, 512], f32, tag="av")
                            for t in range(maxkt):
                                ql = max(0, t * 128 - q0g) if causal else 0
                                nc.tensor.matmul(
                                    av[:, ql:512], vsb[g][:, t, :],
                                    probs[:, t, ql:512],
                                    start=(t == 0), stop=(t == maxkt - 1),
                                    skip_group_check=True)
                            # denominators: probs-stationary, sequential chains
                            c0 = r * 4
                            for m in range(4):
                                tmax = min(maxkt, 4 * qsb + m + 1) if causal else SB
                                for t in range(tmax):
                                    nc.tensor.matmul(
                                        pds[:, c0 + m:c0 + m + 1],
                                        probs[:, t, m * 128:(m + 1) * 128],
                                        onec,
                                        start=(t == 0), stop=(t == tmax - 1),
                                        skip_group_check=True)
                            rrs = dnpool.tile([128, 4], bf16, tag="rrs")
                            with nc.allow_low_precision(reason="softmax recip"):
                                nc.vector.reciprocal(
                                    out=rrs, in_=pds[:, c0:c0 + 4])
                            rrT = prtp.tile([128, 1024], bf16, tag="rrT")
                            nc.tensor.transpose(rrT[0:4, 0:128], rrs, ident)
                            rrc = dnpool.tile([4, 128], bf16, tag="rrc")
                            nc.vector.tensor_copy(out=rrc, in_=rrT[0:4, 0:128])
                            rsb = prsp.tile([128, 512], f32, tag="rsb")
                            for m in range(4):
                                nc.tensor.matmul(
                                    rsb[:, m * 128:(m + 1) * 128],
                                    selt[:, m, :], rrc, start=True, stop=True)
                            rsbs = dnpool.tile([128, 512], bf16, tag="rsbs")
                            nc.scalar.copy(out=rsbs, in_=rsb)
                            oproj_chunk()
                            # fused normalize: att = av * (1/den32)
                            nc.vector.tensor_tensor(
                                out=att[:, h, :], in0=av, in1=rsbs,
                                op=ALU.mult)
                    pending.extend((qsb, att, m) for m in range(KT))
                # drain tail, alternating PSUM banks to double-buffer
                i = 0
                while pending:
                    oproj_chunk(use_psc=(i % 2 == 1))
                    i += 1

    nc.compile()
    return nc


def _get_nc(causal: bool):
    if causal not in _compiled:
        _compiled[causal] = _build(causal)
    return _compiled[causal]


def _split8(a):
    import ml_dtypes
    E4 = ml_dtypes.float8_e4m3
    hi = a.astype(E4)
    lo = (a - hi.astype(np.float32)).astype(E4)
    return hi, lo


def kernel(x, freqs_cis, mask, wq, wk, wv, wo):
    import ml_dtypes
    from concourse.bass_utils import run_bass_kernel_spmd
    BF = ml_dtypes.bfloat16

    x = np.asarray(x, dtype=np.float32)
    freqs_cis = np.asarray(freqs_cis, dtype=np.float32)
    mask = np.asarray(mask, dtype=np.float32)
    wq = np.asarray(wq, dtype=np.float32)
    wk = np.asarray(wk, dtype=np.float32)
    wv = np.asarray(wv, dtype=np.float32)
    wo = np.asarray(wo, dtype=np.float32)

    tri = np.tril(np.ones((S, S), dtype=bool))
    causal = bool((mask[tri] == 0.0).all() and (mask[~tri] < -1e30).all())
    if not causal and not (mask == 0.0).all():
        return _numpy_ref(x, freqs_cis, mask, wq, wk, wv, wo)

    nc = _get_nc(causal)

    cos = freqs_cis[:, :, 0] / WS
    sin = freqs_cis[:, :, 1] / WS
    cosS = np.ascontiguousarray(cos.reshape(SB, 128, 64).transpose(1, 0, 2))
    sinS = np.ascontiguousarray(sin.reshape(SB, 128, 64).transpose(1, 0, 2))
    mtile = (np.ascontiguousarray(mask[0:128, 0:128].T) if causal
             else np.zeros((128, 128), dtype=np.float32))
    ones32 = np.full((128, 1), WS, dtype=BF)
    selS = np.zeros((4, 4, 128), dtype=BF)
    for m in range(4):
        selS[m, m, :] = 1.0
    selS = selS.reshape(4, 512)

    def pack2(a, b):  # [D, E], [D, E] -> [2, D, E]
        return np.ascontiguousarray(np.stack([a, b], axis=0))

    in_maps = []
    for c in range(8):
        b, i = c // 2, c % 2
        xh, xl = _split8(x[b].T)
        wqh, wql = _split8(wq[1024 * i:1024 * (i + 1), :].T * WS)
        wkvf = np.concatenate(
            [wk[256 * i:256 * (i + 1), :].T,
             wv[256 * i:256 * (i + 1), :].T], axis=1) * WS
        wkh, wkl = _split8(wkvf)
        in_maps.append({
            "xT8": pack2(xh, xl),
            "wq8": pack2(wql, wqh),    # weights: [:,0,:]=lo, [:,1,:]=hi
            "wkv8": pack2(wkl, wkh),
            "wob": np.ascontiguousarray(wo[:, 1024 * i:1024 * (i + 1)].T).astype(BF),
            "cosS": cosS, "sinS": sinS, "mtile": mtile,
            "ones32": ones32, "selS": selS,
        })

    res = run_bass_kernel_spmd(nc, in_maps, core_ids=list(range(8)))
    out = np.empty((B, S, D), dtype=np.float32)
    for b in range(B):
        out[b] = res.results[2 * b]["outT"].T + res.results[2 * b + 1]["outT"].T
    return out


def _numpy_ref(x, freqs_cis, mask, wq, wk, wv, wo):
    xq = (x @ wq.T).reshape(B, S, H, HD)
    xk = (x @ wk.T).reshape(B, S, KV, HD)
    xv = (x @ wv.T).reshape(B, S, KV, HD)

    def rope(xh):
        x2 = xh.reshape(*xh.shape[:-1], HD // 2, 2)
        fc = freqs_cis[None, :, None, :, :]
        real = x2[..., 0] * fc[..., 0] - x2[..., 1] * fc[..., 1]
        imag = x2[..., 0] * fc[..., 1] + x2[..., 1] * fc[..., 0]
        return np.concatenate([real, imag], axis=-1)

    xq, xk = rope(xq), rope(xk)
    q = xq.reshape(B, S, KV, NREP, HD)
    sc = np.einsum('bqgrd,bkgd->bgrqk', q, xk) * SCALE + mask[None, None, None]
    sc = sc - sc.max(axis=-1, keepdims=True)
    p = np.exp(sc)
    p /= p.sum(axis=-1, keepdims=True)
    o = np.einsum('bgrqk,bkgd->bqgrd', p, xv).reshape(B, S, H * HD)
    return (o @ wo.T).astype(np.float32)


# revision 23
# speedup vs baseline: 1.3165x; 1.0641x over previous
"""Trainium2 Bass kernel for nn_Attention (B=4, S=2048, D=2048, H=16, KV=4, HD=128).

Sharding (8 cores): data-parallel over batch (4) x tensor-parallel over
KV-head-group halves (2). Core c handles batch b=c//2 and q-heads
[8*(c%2), 8*(c%2)+8) == kv groups {2*(c%2), 2*(c%2)+1}. Each core produces a
partial output (its heads' contribution through wo); the host sums the two
partials per batch.

v5: Q/K/V projections run as fp8e4 DoubleRow matmuls (0.5 cycles/row, two
128-deep contraction subtiles per instruction) with an error-corrected hi/lo
split of both x and the weights (hi*hi + both cross terms; ~2^-8 effective
precision). Weights are pre-scaled x32 on the host so the lo residual clears
e4m3's subnormal floor; the descale folds into cos/sin (q,k) and a 32.0
ones-column in the softmax denominator (v). Attention (scores, exp, AV) and
the output projection run in bf16. Softmax denominators come from
probs-stationary x ones-moving matmuls (out [128q,1], ~1 cycle each instead
of 512); the reciprocal row reaches broadcast layout via a tiny transpose
into the same PSUM bank (lazy region-zeroing verified on HW) + selector
matmuls. Normalization is fused into the PSUM->SBUF copy of AV.

The projection work is cut into per-s-block "units" (three 4-head matmul
pieces + a transpose group) that are interleaved, two per attention rep,
into the PREVIOUS q-superblock's attention stream: attention is exp(ACT)-
paced, so the projection matmuls fill the PE gaps that the in-order engine
could not otherwise skip past. Out-projection chunks are similarly
interleaved one superblock behind. Causal attention for superblock Q only
needs projections of s-blocks < 4(Q+1), which this schedule guarantees.
"""
import numpy as np

B, S, D = 4, 2048, 2048
H, KV, HD = 16, 4, 128
NREP = H // KV
SCALE = float(HD) ** -0.5
WS = 32.0                  # host weight pre-scale (power of 2)

SB = S // 128          # 16 s-blocks
KT = D // 128          # 16 contraction tiles for projections
QSB = S // 512         # 4 q-superblocks
HPC = 8                # q heads per core
GPC = 2                # kv groups per core

_compiled = {}


def _build(causal: bool):
    from collections import deque
    from functools import partial

    import concourse.bass as bass  # noqa: F401
    import concourse.tile as tile
    from concourse import bacc, mybir
    from concourse.masks import make_identity

    f32 = mybir.dt.float32
    bf16 = mybir.dt.bfloat16
    f8 = mybir.dt.float8e4
    DR = mybir.MatmulPerfMode.DoubleRow
    AF = mybir.ActivationFunctionType
    ALU = mybir.AluOpType

    nc = bacc.Bacc("TRN2")

    # x hi/lo: [0]=hi, [1]=lo.  weights hi/lo: [0]=LO, [1]=HI (two-major layout)
    xT8 = nc.dram_tensor("xT8", [2, D, S], f8, kind="ExternalInput")
    wq8 = nc.dram_tensor("wq8", [2, D, HPC * HD], f8, kind="ExternalInput")
    wkv8 = nc.dram_tensor("wkv8", [2, D, 2 * GPC * HD], f8, kind="ExternalInput")
    wob = nc.dram_tensor("wob", [HPC * HD, D], bf16, kind="ExternalInput")
    cosS = nc.dram_tensor("cosS", [128, SB, 64], f32, kind="ExternalInput")
    sinS = nc.dram_tensor("sinS", [128, SB, 64], f32, kind="ExternalInput")
    mtile = nc.dram_tensor("mtile", [128, 128], f32, kind="ExternalInput")
    ones32 = nc.dram_tensor("ones32", [128, 1], bf16, kind="ExternalInput")
    selS = nc.dram_tensor("selS", [4, 4 * 128], bf16, kind="ExternalInput")
    outT = nc.dram_tensor("outT", [D, S], bf16, kind="ExternalOutput")

    xT4 = xT8.rearrange("two (kt p) s -> p two kt s", p=128)
    wq4 = wq8.rearrange("two (kt p) e -> p two kt e", p=128)
    wkv4 = wkv8.rearrange("two (kt p) e -> p two kt e", p=128)
    wo3 = wob.rearrange("(h p) d -> p h d", p=128)

    EW = (HPC + 2 * GPC) * HD  # 1536 projection columns: q 0:1024, kv 1024:1536
    NR = HPC + GPC             # 10 rope heads

    with tile.TileContext(nc) as tc:
        with tc.tile_pool(name="persist", bufs=1) as persist, \
             tc.tile_pool(name="w1", bufs=1) as wpool, \
             tc.tile_pool(name="xs1", bufs=2) as xpool, \
             tc.tile_pool(name="rs1", bufs=2) as rpool, \
             tc.tile_pool(name="tt1", bufs=1) as tpool, \
             tc.tile_pool(name="pr2", bufs=2) as prpool, \
             tc.tile_pool(name="att2", bufs=2) as attpool, \
             tc.tile_pool(name="dn2", bufs=2) as dnpool, \
             tc.tile_pool(name="o2", bufs=2) as opool, \
             tc.tile_pool(name="psc", bufs=3, space="PSUM") as pscp, \
             tc.tile_pool(name="ps1", bufs=1, space="PSUM") as ps1p, \
             tc.tile_pool(name="pav", bufs=1, space="PSUM") as pavp, \
             tc.tile_pool(name="pds", bufs=1, space="PSUM") as pdsp, \
             tc.tile_pool(name="prs", bufs=1, space="PSUM") as prsp, \
             tc.tile_pool(name="ppt", bufs=1, space="PSUM") as ptp:
            # per-qsb-chunk persistent tiles (dep granularity for interleave)
            qT = [[persist.tile([128, 512], bf16, tag=f"qT{h}_{c}",
                                name=f"qT{h}_{c}")
                   for c in range(QSB)] for h in range(HPC)]
            kT = [[persist.tile([128, 512], bf16, tag=f"kT{g}_{c}",
                                name=f"kT{g}_{c}")
                   for c in range(QSB)] for g in range(GPC)]
            vsb = [[persist.tile([128, 4, 128], bf16, tag=f"v{g}_{c}",
                    name=f"v{g}_{c}")
                    for c in range(QSB)] for g in range(GPC)]
            wos = persist.tile([128, HPC, D], bf16, tag="wos")
            msk = persist.tile([128, 128], f32, tag="msk")
            nc.sync.dma_start(out=msk, in_=mtile[:, :])
            onec = persist.tile([128, 1], bf16, tag="onec")
            nc.sync.dma_start(out=onec, in_=ones32[:, :])
            selt = persist.tile([4, 4, 128], bf16, tag="selt")
            sel3 = selS.rearrange("k (m d) -> k m d", d=128)
            nc.sync.dma_start(out=selt, in_=sel3[:, :, :])
            cos_t = persist.tile([128, SB, 64], f32, tag="cos")
            sin_t = persist.tile([128, SB, 64], f32, tag="sin")
            nc.sync.dma_start(out=cos_t, in_=cosS[:, :, :])
            nc.sync.dma_start(out=sin_t, in_=sinS[:, :, :])
            ident = persist.tile([128, 128], bf16, tag="ident")
            ident_f = persist.tile([128, 128], f32, tag="identf")
            make_identity(nc, ident_f)
            nc.vector.tensor_copy(out=ident, in_=ident_f)

            wtq = wpool.tile([128, 2, KT, HPC * HD], f8, tag="wtq")
            wtk = wpool.tile([128, 2, KT, 2 * GPC * HD], f8, tag="wtk")
            # prefetch sb0's x before the weights so compute can stream
            xs0 = xpool.tile([128, 2, KT, 128], f8, tag="xs")
            for v in range(2):
                nc.sync.dma_start(out=xs0[:, v, 0:8, :],
                                  in_=xT4[:, v, 0:8, 0:128])
                nc.sync.dma_start(out=xs0[:, v, 8:16, :],
                                  in_=xT4[:, v, 8:16, 0:128])
            # weights interleaved by kt-pair: sb0's kt-streamed chains
            # consume each pair as it lands
            for k0 in range(0, KT, 2):
                for v in range(2):
                    nc.sync.dma_start(out=wtq[:, v, k0:k0 + 2, :],
                                      in_=wq4[:, v, k0:k0 + 2, :])
                for v in range(2):
                    nc.sync.dma_start(out=wtk[:, v, k0:k0 + 2, :],
                                      in_=wkv4[:, v, k0:k0 + 2, :])

            def wsl_hi(kt0, kt1, n0):  # [128, 2(kt), 256] hi slice
                if n0 < HPC * HD:
                    return wtq[:, 1, kt0:kt1, n0:n0 + 256]
                n0 -= HPC * HD
                return wtk[:, 1, kt0:kt1, n0:n0 + 256]

            def wsl_x(kt, n0):  # [128, 2(lo,hi), 256] cross slice
                if n0 < HPC * HD:
                    return wtq[:, 0:2, kt, n0:n0 + 256]
                n0 -= HPC * HD
                return wtk[:, 0:2, kt, n0:n0 + 256]

            # ---------------- stage-1 units ----------------
            s1state = {}

            def emit_piece(sb, ti):
                if ti == 0:
                    if sb in (3, 6, 9, 12):
                        c = {3: 0, 6: 1, 9: 2, 12: 3}[sb]
                        nc.sync.dma_start(
                            out=wos[:, :, 512 * c:512 * (c + 1)],
                            in_=wo3[:, :, 512 * c:512 * (c + 1)])
                    if sb == 0:
                        xs = xs0
                    else:
                        xs = xpool.tile([128, 2, KT, 128], f8, tag="xs")
                        for v in range(2):
                            nc.sync.dma_start(
                                out=xs[:, v, 0:8, :],
                                in_=xT4[:, v, 0:8, sb * 128:(sb + 1) * 128])
                            nc.sync.dma_start(
                                out=xs[:, v, 8:16, :],
                                in_=xT4[:, v, 8:16, sb * 128:(sb + 1) * 128])
                    rp = rpool.tile([128, NR, 128], bf16, tag="rope")
                    s1state[sb] = (xs, rp)
                xs, rp = s1state[sb]
                ps = ps1p.tile([128, 512], f32, tag="ps1")
                for half in (0, 1):
                    n0 = ti * 512 + half * 256
                    lo = half * 256
                    for j in range(KT // 2):
                        nc.tensor.matmul(
                            ps[:, lo:lo + 256],
                            xs[:, 0, 2 * j:2 * j + 2, :],
                            wsl_hi(2 * j, 2 * j + 2, n0),
                            start=(j == 0), stop=False, perf_mode=DR,
                            skip_group_check=True)
                    for kt in range(KT):
                        nc.tensor.matmul(
                            ps[:, lo:lo + 256],
                            xs[:, 0:2, kt, :],
                            wsl_x(kt, n0),
                            start=False, stop=(kt == KT - 1),
                            perf_mode=DR, skip_group_check=True)
                p3 = ps.rearrange("p (h d) -> p h d", d=128)
                h0 = ti * 4
                nr = 2 if ti == 2 else 4
                if ti == 2:
                    c = sb // 4
                    for g in range(GPC):
                        nc.scalar.copy(out=vsb[g][c][:, sb % 4, :],
                                       in_=p3[:, 2 + g, :])
                ev = p3[:, 0:nr, 0:128:2]
                od = p3[:, 0:nr, 1:128:2]
                cb = cos_t[:, None, sb, :].broadcast_to([128, nr, 64])
                sn = sin_t[:, None, sb, :].broadcast_to([128, nr, 64])
                t1 = rpool.tile([128, 4, 64], f32, tag="t1")
                t2 = rpool.tile([128, 4, 64], f32, tag="t2")
                nc.vector.tensor_tensor(
                    out=t1[:, 0:nr, :], in0=ev, in1=cb, op=ALU.mult)
                nc.vector.tensor_tensor(
                    out=t2[:, 0:nr, :], in0=od, in1=sn, op=ALU.mult)
                nc.vector.tensor_tensor(
                    out=rp[:, h0:h0 + nr, 0:64], in0=t1[:, 0:nr, :],
                    in1=t2[:, 0:nr, :], op=ALU.subtract)
                nc.vector.tensor_tensor(
                    out=t1[:, 0:nr, :], in0=ev, in1=sn, op=ALU.mult)
                nc.vector.tensor_tensor(
                    out=t2[:, 0:nr, :], in0=od, in1=cb, op=ALU.mult)
                nc.vector.tensor_tensor(
                    out=rp[:, h0:h0 + nr, 64:128], in0=t1[:, 0:nr, :],
                    in1=t2[:, 0:nr, :], op=ALU.add)

            def emit_transposes(sb):
                xs, rp = s1state.pop(sb)
                c = sb // 4
                col = (sb % 4) * 128
                pt = ptp.tile([128, 1024], bf16, tag="pt")
                for h in range(NR):
                    reg = (h % 8) * 128
                    nc.tensor.transpose(
                        pt[:, reg:reg + 128], rp[:, h, :], ident)
                    dst = qT[h][c] if h < HPC else kT[h - HPC][c]
                    nc.scalar.copy(out=dst[:, col:col + 128],
                                   in_=pt[:, reg:reg + 128])

            unitq = deque()

            def queue_quad(q):
                for sb in range(4 * q, 4 * q + 4):
                    for ti in range(3):
                        unitq.append(partial(emit_piece, sb, ti))
                    unitq.append(partial(emit_transposes, sb))

            pending = []  # (qsb, att_tile, m) out-proj chunks not yet run

            def oproj_chunk():
                if not pending:
                    return
                oq, oatt, m = pending.pop(0)
                po = pscp.tile([128, 512], f32, tag="sc")
                for e in range(HPC):
                    nc.tensor.matmul(
                        po, wos[:, e, m * 128:(m + 1) * 128],
                        oatt[:, e, :], start=(e == 0), stop=(e == HPC - 1),
                        skip_group_check=True)
                ot = opool.tile([128, 512], bf16, tag="ot")
                nc.vector.tensor_copy(out=ot, in_=po)
                nc.sync.dma_start(
                    out=outT[m * 128:(m + 1) * 128, oq * 512:(oq + 1) * 512],
                    in_=ot)

            def slot():
                if unitq:
                    unitq.popleft()()
                oproj_chunk()

            def attention(qsb):
                att = attpool.tile([128, HPC, 512], bf16, tag="att")
                maxkt = (qsb + 1) * 4 if causal else SB
                q0g = qsb * 512
                for g in range(GPC):
                    pds = pdsp.tile([128, 512], f32, tag="pds")
                    for r in range(NREP):
                        h = g * NREP + r
                        probs = prpool.tile([128, SB, 512], bf16, tag="probs")
                        for t in range(maxkt):
                            ql = max(0, t * 128 - q0g) if causal else 0
                            sc = pscp.tile([128, 512], f32, tag="sc")
                            nc.tensor.matmul(
                                sc[:, ql:512],
                                kT[g][t // 4][:, (t % 4) * 128:(t % 4 + 1) * 128],
                                qT[h][qsb][:, ql:512],
                                start=True, stop=True)
                            if causal and t * 128 >= q0g:
                                nc.vector.tensor_tensor(
                                    out=sc[:, ql:ql + 128],
                                    in0=sc[:, ql:ql + 128],
                                    in1=msk, op=ALU.add)
                            nc.scalar.activation(
                                out=probs[:, t, ql:512],
                                in_=sc[:, ql:512], func=AF.Exp,
                                scale=SCALE)
                        slot()
                        # AV accumulation (x WS via v scaling)
                        av = pavp.tile([128, 512], f32, tag="av")
                        for t in range(maxkt):
                            ql = max(0, t * 128 - q0g) if causal else 0
                            nc.tensor.matmul(
                                av[:, ql:512],
                                vsb[g][t // 4][:, t % 4, :],
                                probs[:, t, ql:512],
                                start=(t == 0), stop=(t == maxkt - 1),
                                skip_group_check=True)
                        # denominators: probs-stationary, sequential chains
                        c0 = r * 4
                        for m in range(4):
                            tmax = min(maxkt, 4 * qsb + m + 1) if causal else SB
                            for t in range(tmax):
                                nc.tensor.matmul(
                                    pds[:, c0 + m:c0 + m + 1],
                                    probs[:, t, m * 128:(m + 1) * 128],
                                    onec,
                                    start=(t == 0), stop=(t == tmax - 1),
                                    skip_group_check=True)
                        rrs = dnpool.tile([128, 4], f32, tag="rrs")
                        with nc.allow_low_precision(reason="softmax recip"):
                            nc.vector.reciprocal(out=rrs, in_=pds[:, c0:c0 + 4])
                        # transpose into the dsum bank (safe: no chain is
                        # mid-flight in it here; lazy region zeroing)
                        nc.tensor.transpose(pds[0:4, 16:144], rrs, ident_f)
                        rrc = dnpool.tile([4, 128], bf16, tag="rrc")
                        nc.vector.tensor_copy(out=rrc, in_=pds[0:4, 16:144])
                        rsb = prsp.tile([128, 512], f32, tag="rsb")
                        for m in range(4):
                            nc.tensor.matmul(
                                rsb[:, m * 128:(m + 1) * 128],
                                selt[:, m, :], rrc, start=True, stop=True)
                        rsbs = dnpool.tile([128, 512], bf16, tag="rsbs")
                        nc.scalar.copy(out=rsbs, in_=rsb)
                        slot()
                        # fused normalize: att = av * (1/den32)
                        nc.vector.tensor_tensor(
                            out=att[:, h, :], in0=av, in1=rsbs, op=ALU.mult)
                pending.extend((qsb, att, m) for m in range(KT))

            # ---------------- schedule ----------------
            queue_quad(0)
            if not causal:
                for q in range(1, QSB):
                    queue_quad(q)
            while unitq:  # quad 0 (or all, if non-causal) runs up front
                unitq.popleft()()
            for qsb in range(QSB):
                if causal and qsb + 1 < QSB:
                    queue_quad(qsb + 1)
                attention(qsb)
                while unitq:  # safety: next quad must be done before its attn
                    unitq.popleft()()
            while pending:  # drain last superblock's out-projection
                oproj_chunk()

    nc.compile()
    return nc


def _get_nc(causal: bool):
    if causal not in _compiled:
        _compiled[causal] = _build(causal)
    return _compiled[causal]


def _split8(a):
    import ml_dtypes
    E4 = ml_dtypes.float8_e4m3
    hi = a.astype(E4)
    lo = (a - hi.astype(np.float32)).astype(E4)
    return hi, lo


def kernel(x, freqs_cis, mask, wq, wk, wv, wo):
    import ml_dtypes
    from concourse.bass_utils import run_bass_kernel_spmd
    BF = ml_dtypes.bfloat16

    x = np.asarray(x, dtype=np.float32)
    freqs_cis = np.asarray(freqs_cis, dtype=np.float32)
    mask = np.asarray(mask, dtype=np.float32)
    wq = np.asarray(wq, dtype=np.float32)
    wk = np.asarray(wk, dtype=np.float32)
    wv = np.asarray(wv, dtype=np.float32)
    wo = np.asarray(wo, dtype=np.float32)

    tri = np.tril(np.ones((S, S), dtype=bool))
    causal = bool((mask[tri] == 0.0).all() and (mask[~tri] < -1e30).all())
    if not causal and not (mask == 0.0).all():
        return _numpy_ref(x, freqs_cis, mask, wq, wk, wv, wo)

    nc = _get_nc(causal)

    cos = freqs_cis[:, :, 0] / WS
    sin = freqs_cis[:, :, 1] / WS
    cosS = np.ascontiguousarray(cos.reshape(SB, 128, 64).transpose(1, 0, 2))
    sinS = np.ascontiguousarray(sin.reshape(SB, 128, 64).transpose(1, 0, 2))
    mtile = (np.ascontiguousarray(mask[0:128, 0:128].T) if causal
             else np.zeros((128, 128), dtype=np.float32))
    ones32 = np.full((128, 1), WS, dtype=BF)
    selS = np.zeros((4, 4, 128), dtype=BF)
    for m in range(4):
        selS[m, m, :] = 1.0
    selS = selS.reshape(4, 512)

    def pack2(a, b):  # [D, E], [D, E] -> [2, D, E]
        return np.ascontiguousarray(np.stack([a, b], axis=0))

    in_maps = []
    for c in range(8):
        b, i = c // 2, c % 2
        xh, xl = _split8(x[b].T)
        wqh, wql = _split8(wq[1024 * i:1024 * (i + 1), :].T * WS)
        wkvf = np.concatenate(
            [wk[256 * i:256 * (i + 1), :].T,
             wv[256 * i:256 * (i + 1), :].T], axis=1) * WS
        wkh, wkl = _split8(wkvf)
        in_maps.append({
            "xT8": pack2(xh, xl),
            "wq8": pack2(wql, wqh),    # weights: [0]=lo, [1]=hi
            "wkv8": pack2(wkl, wkh),
            "wob": np.ascontiguousarray(
                wo[:, 1024 * i:1024 * (i + 1)].T).astype(BF),
            "cosS": cosS, "sinS": sinS, "mtile": mtile,
            "ones32": ones32, "selS": selS,
        })

    res = run_bass_kernel_spmd(nc, in_maps, core_ids=list(range(8)))
    out = np.empty((B, S, D), dtype=np.float32)
    for b in range(B):
        out[b] = (res.results[2 * b]["outT"].T.astype(np.float32)
                  + res.results[2 * b + 1]["outT"].T.astype(np.float32))
    return out


def _numpy_ref(x, freqs_cis, mask, wq, wk, wv, wo):
    xq = (x @ wq.T).reshape(B, S, H, HD)
    xk = (x @ wk.T).reshape(B, S, KV, HD)
    xv = (x @ wv.T).reshape(B, S, KV, HD)

    def rope(xh):
        x2 = xh.reshape(*xh.shape[:-1], HD // 2, 2)
        fc = freqs_cis[None, :, None, :, :]
        real = x2[..., 0] * fc[..., 0] - x2[..., 1] * fc[..., 1]
        imag = x2[..., 0] * fc[..., 1] + x2[..., 1] * fc[..., 0]
        return np.concatenate([real, imag], axis=-1)

    xq, xk = rope(xq), rope(xk)
    q = xq.reshape(B, S, KV, NREP, HD)
    sc = np.einsum('bqgrd,bkgd->bgrqk', q, xk) * SCALE + mask[None, None, None]
    sc = sc - sc.max(axis=-1, keepdims=True)
    p = np.exp(sc)
    p /= p.sum(axis=-1, keepdims=True)
    o = np.einsum('bgrqk,bkgd->bqgrd', p, xv).reshape(B, S, H * HD)
    return (o @ wo.T).astype(np.float32)
